# revision 6
# baseline (speedup 1.0000x reference)
"""Trainium2 Bass kernel for the MAB (multihead attention block) problem.

Full inputs in, full outputs out. Data-parallel over batch: 16 batches
across 8 NeuronCores = 2 batches/core. No collectives.

Key techniques vs the bf16 baseline:
  * Attention matmuls (scores S^T = Kp Qp^T and PV) run in fp8e4 with
    MatmulPerfMode.DoubleRow -> 2 output columns/cycle (2x bf16 rate).
    Q/K are scaled by 4 before the fp8 cast (the 1/16 folds into the
    softmax exp scale); V is scaled by 4 with a 4.0-ones column so the
    denominator row cancels the factor exactly.
  * Scores layout: Wq/Wk columns are permuted on the host so the
    projection emits Qp^T/Kp^T with each head's 64 dims split as
    [32 partitions x 2 free planes] - the layout DoubleRow contracts
    over. All downstream tensors stay in this permuted-D order (LayerNorm
    is permutation invariant; Wo rows+cols are permuted to match) and the
    final output DMA un-permutes via a strided access pattern.
  * exp: split between the ACT engine (Exp activation, fp8e4 output,
    ~1 elem/lane/cycle) and the DVE (fast-exp: round(x*c1+c2) into a
    uint8 buffer whose bits are read as fp8e4; the softmax normalization
    cancels the systematic bias of the linear-mantissa approximation).
  * ACT runs only Exp-table functions (exp, copy); LN rsqrt is a
    Quake-style bit hack + 2 Newton steps on the DVE, so there are no
    activation-table swaps.
  * LN applies run on the otherwise-idle GpSimd engine (SBUF->SBUF),
    as does the Qp bf16 -> fp8 quantize.
  * FFN relu+residual is one fused DVE scalar_tensor_tensor
    ((f max 0) + ln1).
"""

import math
import sys
from contextlib import ExitStack

import numpy as np

sys.path.insert(0, "/opt/trn_rl_repo")

import concourse.bass as bass
import concourse.tile as tile
from concourse import bacc
from concourse import mybir
from concourse.bass import ds, ts
from concourse.bass_utils import run_bass_kernel_spmd
from concourse.masks import make_identity

FP = mybir.dt.float32
BF = mybir.dt.bfloat16
F8 = mybir.dt.float8e4
U8 = mybir.dt.uint8
I32 = mybir.dt.int32
AF = mybir.ActivationFunctionType
ALU = mybir.AluOpType
DR = mybir.MatmulPerfMode.DoubleRow

B, N, D = 16, 1024, 512
NCORES = 8
BL = B // NCORES  # batches per core
H, HD = 8, 64
SCALE = 1.0 / math.sqrt(D)
EPS = 1e-5
P = 128
DT = D // P  # 4 dv chunks
NT = N // P  # 8 nq/nk tiles
HA = HD + 1  # head dim + denominator column
HB = HA + 1  # 66: pad so bf16 PSUM blocks stay 4B-aligned
LOG2E = 1.4426950408889634
# fast-exp: fp8e4 bits of exp(x*SCALE/16) ~= round(x*FE_C1 + FE_C2) as u8
FE_C1 = 8.0 * LOG2E * (SCALE / 16.0)
FE_C2 = 55.6
# which (head, m) exp tiles run on DVE as fast-exp (the rest on ACT)
FASTEXP_DVE = {(h, 5) for h in range(H)} | {(1, 2), (5, 2)}


def _bcast_ap(ap):
    """Broadcast a [D]-shaped DRAM AP across all 128 partitions."""
    return bass.AP(tensor=ap.tensor, offset=ap.offset, ap=[[0, P]] + list(ap.ap))


def _free_bcast(tileap, inner):
    """[P, K] tile viewed as [P, K, inner] with stride-0 inner dim."""
    return bass.AP(
        tensor=tileap.tensor,
        offset=tileap.offset,
        ap=[list(tileap.ap[0]), list(tileap.ap[1]), [0, inner]],
    )


def _build_program(triv0, triv1, trivbo):
    nc = bacc.Bacc(None, target_bir_lowering=False)
    dr = {}
    for name, shape in [
        ("QT", [BL, D, N]),
        ("KT", [BL, D, N]),
        ("Wq", [D, D]),   # columns pre-permuted on host
        ("Wk", [D, D]),   # columns pre-permuted on host
        ("Wv", [D, D]),
        ("Wo", [D, D]),   # rows+columns pre-permuted on host
        ("bq2", [P, DT]),
        ("bk2", [P, DT]),
        ("bv4", [D]),     # 4 * bv
        ("bo", [D]),      # permuted
        ("g0", [D]),      # permuted
        ("b0", [D]),      # permuted
        ("g1", [D]),      # permuted
        ("b1", [D]),      # permuted
    ]:
        dt = BF if name in ("QT", "KT", "Wq", "Wk", "Wv", "Wo") else FP
        dr[name] = nc.declare_dram_parameter(name, shape, dt, isOutput=False)
    out_O = nc.declare_dram_parameter("O", [BL, N, D], FP, isOutput=True)

    qt_src = dr["QT"][:].rearrange("b (c p) n -> b p c n", p=P)
    kt_src = dr["KT"][:].rearrange("b (c p) n -> b p c n", p=P)

    with tile.TileContext(nc) as tc, ExitStack() as ctx:
        singles = ctx.enter_context(tc.tile_pool(name="singles", bufs=1))
        work = ctx.enter_context(tc.tile_pool(name="work", bufs=1))
        pch = ctx.enter_context(tc.tile_pool(name="pch", bufs=3))
        lnt = ctx.enter_context(tc.tile_pool(name="lnt", bufs=2))
        ost = ctx.enter_context(tc.tile_pool(name="ost", bufs=2))
        otile = ctx.enter_context(tc.tile_pool(name="otile", bufs=2))
        sml = ctx.enter_context(tc.tile_pool(name="sml", bufs=8))
        # PSUM: flow 2x2 banks (score tiles) + opv 2 + acc 2x1 = 8 banks
        ps_acc = ctx.enter_context(tc.tile_pool(name="ps_acc", bufs=2, space="PSUM"))
        ps_pv = ctx.enter_context(tc.tile_pool(name="ps_pv", bufs=1, space="PSUM"))
        ps_flow = ctx.enter_context(tc.tile_pool(name="ps_flow", bufs=2, space="PSUM"))

        # ---- statics
        wsb = {}
        for wname in ("Wq", "Wk", "Wv", "Wo"):
            wsb[wname] = singles.tile([P, DT, D], BF, tag=wname, name=wname)
        nc.sync.dma_start(
            out=wsb["Wq"], in_=dr["Wq"][:].rearrange("(c p) d -> p c d", p=P)
        )
        bq_sb = singles.tile([P, DT], FP, tag="bq2")
        nc.sync.dma_start(out=bq_sb, in_=dr["bq2"][:])
        bk_sb = singles.tile([P, DT], FP, tag="bk2")
        bc = {}
        for bname in ("bv4", "bo", "g0", "b0", "g1", "b1"):
            t = singles.tile([P, D], FP, tag=bname)
            nc.gpsimd.dma_start(out=t, in_=_bcast_ap(dr[bname][:]))
            bc[bname] = t
        ident = singles.tile([P, P], FP, tag="ident")
        make_identity(nc, ident)
        ident_b = singles.tile([P, P], BF, tag="identb")
        nc.vector.tensor_copy(ident_b, ident)

        state = {}

        def rsqrt_dve(out_ap, in_ap, scratch_pool, n):
            """out = 1/sqrt(in) via bit hack + 2 Newton iters. [P, n] fp32."""
            yh = scratch_pool.tile([P, n], FP, tag=f"rs_a{n}", name="rs_a")
            t0 = scratch_pool.tile([P, n], FP, tag=f"rs_b{n}", name="rs_b")
            nc.vector.tensor_scalar(
                out=yh[:].bitcast(I32), in0=in_ap.bitcast(I32),
                scalar1=1, scalar2=None, op0=ALU.logical_shift_right,
            )
            nc.vector.tensor_scalar(
                out=out_ap.bitcast(I32), in0=yh[:].bitcast(I32),
                scalar1=-1, scalar2=0x5F3759DF, op0=ALU.mult, op1=ALU.add,
            )
            for _ in range(2):
                nc.vector.tensor_tensor(t0, in_ap, out_ap, ALU.mult)
                nc.vector.tensor_tensor(t0, t0, out_ap, ALU.mult)
                nc.vector.tensor_scalar(
                    out=t0, in0=t0, scalar1=-0.5, scalar2=1.5,
                    op0=ALU.mult, op1=ALU.add,
                )
                nc.vector.tensor_tensor(out_ap, out_ap, t0, ALU.mult)

        def phase_a(b):
            st = {}
            qt = work.tile([P, DT, N], BF, tag="qt")
            kt = work.tile([P, DT, N], BF, tag="kt")
            for c in range(DT):
                nc.sync.dma_start(out=qt[:, c, :], in_=qt_src[b, :, c, :])
                nc.sync.dma_start(out=kt[:, c, :], in_=kt_src[b, :, c, :])
            if b == 0:
                for wname in ("Wk", "Wv", "Wo"):
                    nc.sync.dma_start(
                        out=wsb[wname],
                        in_=dr[wname][:].rearrange("(c p) d -> p c d", p=P),
                    )
                nc.sync.dma_start(out=bk_sb, in_=dr["bk2"][:])

            # Qp^T (bf16, for residual+fp8 quantize) and Kp^T (straight to fp8)
            qpt = work.tile([P, DT, N], BF, tag="qpt")
            ks = work.tile([P, DT, N], F8, tag="ks")
            qs = work.tile([P, DT, N], F8, tag="qs")
            for t in range(DT):
                ps = ps_flow.tile([P, N], FP, tag="flow", name="projq")
                for hf in range(2):
                    for c in range(DT):
                        nc.tensor.matmul(
                            ps[:, ds(hf * 512, 512)],
                            wsb["Wq"][:, c, ts(t, P)],
                            qt[:, c, ds(hf * 512, 512)],
                            start=(c == 0),
                            stop=(c == DT - 1),
                        )
                nc.vector.tensor_scalar_add(qpt[:, t, :], ps, bq_sb[:, t : t + 1])
                # fp8 copy (x4) for the score matmuls, on GpSimd
                nc.gpsimd.tensor_scalar_mul(qs[:, t, :], qpt[:, t, :], 4.0)
            for t in range(DT):
                ps = ps_flow.tile([P, N], FP, tag="flow", name="projk")
                for hf in range(2):
                    for c in range(DT):
                        nc.tensor.matmul(
                            ps[:, ds(hf * 512, 512)],
                            wsb["Wk"][:, c, ts(t, P)],
                            kt[:, c, ds(hf * 512, 512)],
                            start=(c == 0),
                            stop=(c == DT - 1),
                        )
                # ks = (Kp + bk) * 4 -> fp8
                nc.vector.tensor_scalar(
                    out=ks[:, t, :], in0=ps, scalar1=bk_sb[:, t : t + 1],
                    scalar2=4.0, op0=ALU.add, op1=ALU.mult,
                )

            # Vp natural, augmented: per head 4*V (64 cols) + a 4.0 column
            vpa = work.tile([P, NT, H * HB], F8, tag="vpa")
            ones_ap = bass.AP(
                tensor=vpa.tensor, offset=vpa.offset + HD,
                ap=[list(vpa.ap[0]), [H * HB, NT], [HB, H], [1, 1]],
            )
            nc.gpsimd.memset(ones_ap, 4.0)

            def emit_vp(m):
                ps = ps_acc.tile([P, 512], FP, tag="acc", name="vps")
                for c in range(DT):
                    nc.tensor.matmul(
                        ps,
                        kt[:, c, ts(m, P)],
                        wsb["Wv"][:, c, :],
                        start=(c == 0),
                        stop=(c == DT - 1),
                    )
                vslice = bass.AP(
                    tensor=vpa.tensor, offset=vpa.offset + m * (H * HB),
                    ap=[list(vpa.ap[0]), [HB, H], [1, HD]],
                )
                # vpa = 4*Vp + 4*bv  (bv4 = 4*bv from host)
                nc.vector.scalar_tensor_tensor(
                    out=vslice,
                    in0=ps[:, :].rearrange("p (h s) -> p h s", s=HD),
                    scalar=4.0,
                    in1=bc["bv4"][:, :].rearrange("p (h s) -> p h s", s=HD),
                    op0=ALU.mult,
                    op1=ALU.add,
                )

            for m in range(4):
                emit_vp(m)
            st_vp_rest = [lambda m=m: emit_vp(m) for m in range(4, NT)]

            # Qp natural-permuted (for the attention residual), bf16
            qp = work.tile([P, NT, D], BF, tag="qp")

            st.update(qt=qt, kt=kt, qpt=qpt, qs=qs, ks=ks, vpa=vpa, qp=qp,
                      vp_rest=st_vp_rest)
            state[b] = st

        def qp_chunk(b, t):
            st = state[b]
            qpt, qp = st["qpt"], st["qp"]
            for half in range(2):
                tp = ps_acc.tile([P, 512], BF, tag="acc", name="qptr")
                for mm in range(4):
                    m = half * 4 + mm
                    nc.tensor.transpose(
                        tp[:, ts(mm, P)], qpt[:, t, ts(m, P)], ident_b
                    )
                nc.any.tensor_copy(
                    qp[:, ds(half * 4, 4), ts(t, P)],
                    tp.rearrange("p (mm n) -> p mm n", n=P),
                )

        def phase_b(b):
            st = state[b]
            qs, ks, vpa, qp = st["qs"], st["ks"], st["vpa"], st["qp"]
            oasm = work.tile([P, NT, D], BF, tag="oasm")
            pending_drain = [None]
            carry = []  # deferred PV matmul thunks crossing group boundary

            for h in range(H):
                g, a = h // 4, h % 4
                prow = ds(32 * a, 32)
                o_pair = ps_pv.tile([HA, N], FP, tag="opv")

                def emit_pv(t, p2, o_pair=o_pair, h=h):
                    lhs = bass.AP(
                        tensor=vpa.tensor,
                        offset=vpa.offset + 2 * t * (H * HB) + h * HB,
                        ap=[list(vpa.ap[0]), [H * HB, 2], [1, HA]],
                    )
                    for half in range(2):
                        nc.tensor.matmul(
                            o_pair[:, ds(half * 512, 512)],
                            lhs,
                            p2[:, :, ds(half * 512, 512)],
                            start=(t == 0),
                            stop=(t == 3),
                            perf_mode=DR,
                        )

                pend = []
                p2 = None
                for m in range(NT):
                    if m % 2 == 0:
                        p2 = pch.tile([P, 2, N], F8, tag="p2")
                    s = ps_flow.tile([P, N], FP, tag="flow", name="spair")
                    for half in range(2):
                        nc.tensor.matmul(
                            s[:, ds(half * 512, 512)],
                            ks[prow, ds(2 * g, 2), ts(m, P)],
                            qs[prow, ds(2 * g, 2), ds(half * 512, 512)],
                            start=True,
                            stop=True,
                            perf_mode=DR,
                            tile_position=(32 * a, 0),
                        )
                    if (h, m) in FASTEXP_DVE:
                        nc.vector.tensor_scalar(
                            out=p2[:, m % 2, :].bitcast(U8),
                            in0=s, scalar1=FE_C1, scalar2=FE_C2,
                            op0=ALU.mult, op1=ALU.add,
                        )
                    else:
                        nc.scalar.activation(
                            p2[:, m % 2, :], s, AF.Exp, scale=SCALE / 16.0
                        )
                    if carry:
                        carry.pop(0)()
                    if h == 0 and m in (0, 1):
                        for _ in range(2):
                            if st["vp_rest"]:
                                st["vp_rest"].pop(0)()
                    if m == 2 and pending_drain[0] is not None:
                        pending_drain[0]()
                        pending_drain[0] = None
                    if m % 2 == 1:
                        pend.append((m // 2, p2))
                        if len(pend) > 1:
                            emit_pv(*pend.pop(0))
                for t, pp in pend:
                    carry.append(lambda t=t, pp=pp, f=emit_pv: f(t, pp))
                pend = []

                # drain: PSUM -> SBUF bf16, transpose to natural, normalize by
                # the denominator row, add the Qp residual. Deferred into the
                # next group so the PE keeps streaming scores.
                def make_drain(o_pair=o_pair, h=h, g=g, a=a):
                    def drain():
                        o_sb = ost.tile([HA, N], BF, tag="ost", name="osb")
                        nc.vector.tensor_copy(o_sb, o_pair)
                        t_ps = ps_acc.tile([P, NT, HB], BF, tag="acc", name="otr")
                        for q in range(NT):
                            nc.tensor.transpose(
                                t_ps[:, q, 0:HA],
                                o_sb[:, ts(q, P)],
                                ident_b[0:HA, 0:HA],
                            )
                        r8 = sml.tile([P, NT], FP, tag="r8", name="r8")
                        den = bass.AP(
                            tensor=t_ps.tensor, offset=t_ps.offset + HD,
                            ap=[list(t_ps.ap[0]), [HB, NT]],
                        )
                        nc.vector.reciprocal(r8, den)
                        tmp = otile.tile([P, NT, HD], BF, tag="dtmp", name="dtmp")
                        nc.vector.tensor_tensor(
                            tmp,
                            bass.AP(
                                tensor=t_ps.tensor, offset=t_ps.offset,
                                ap=[list(t_ps.ap[0]), [HB, NT], [1, HD]],
                            ),
                            _free_bcast(r8[:], HD),
                            ALU.mult,
                        )
                        # head h columns in permuted-D order: two 32-blocks at
                        # 256g+32a and 256g+128+32a, per q tile
                        def hcols(tl):
                            return bass.AP(
                                tensor=tl.tensor,
                                offset=tl.offset + 256 * g + 32 * a,
                                ap=[list(tl.ap[0]), [D, NT], [P, 2], [1, 32]],
                            )
                        nc.vector.tensor_tensor(
                            hcols(oasm),
                            tmp[:].rearrange("p q (i e) -> p q i e", e=32),
                            hcols(qp),
                            ALU.add,
                        )
                    return drain

                pending_drain[0] = make_drain()
                if h == 0:
                    qp_chunk(b, 0)
                    qp_chunk(b, 1)
                elif h in (1, 2):
                    qp_chunk(b, h + 1)

            for f in carry:
                f()
            carry = []
            if pending_drain[0] is not None:
                pending_drain[0]()
                pending_drain[0] = None
            st.update(oasm=oasm)

        def phase_c_ln1(b):
            st = state[b]
            oasm = st["oasm"]
            ln1 = work.tile([P, NT, D], BF, tag="ln1", name="ln1")
            st["ln1"] = ln1
            mva = sml.tile([P, NT, 2], FP, tag="mva", name="mva")
            for q in range(NT):
                st_ = sml.tile([P, 6], FP, tag="bn", name="st")
                nc.vector.bn_stats(st_, oasm[:, q, :])
                nc.vector.bn_aggr(mva[:, q, :], st_)
            veps = sml.tile([P, NT], FP, tag="veps", name="veps")
            nc.vector.tensor_scalar(
                out=veps, in0=mva[:, :, 1], scalar1=EPS, scalar2=None,
                op0=ALU.add,
            )
            rsa = sml.tile([P, NT], FP, tag="rsa", name="rsa")
            rsqrt_dve(rsa[:], veps[:], sml, NT)
            for q in range(NT):
                lq = ln1[:, q, :]
                nc.gpsimd.tensor_scalar(
                    out=lq,
                    in0=oasm[:, q, :],
                    scalar1=mva[:, q, 0:1],
                    scalar2=rsa[:, q : q + 1],
                    op0=ALU.subtract,
                    op1=ALU.mult,
                )
                if not triv0:
                    nc.vector.tensor_tensor(lq, lq, bc["g0"], ALU.mult)
                    nc.vector.tensor_tensor(lq, lq, bc["b0"], ALU.add)

        def phase_c_ffn(b):
            st = state[b]
            ln1 = st["ln1"]
            # unscramble AP pieces for the final output DMA (per g half)
            for q in range(NT):
                lq = ln1[:, q, :]
                tp = ps_flow.tile([P, N], BF, tag="flow", name="lntr")
                for c in range(DT):
                    nc.tensor.transpose(tp[:, ts(c, P)], lq[:, ts(c, P)], ident_b)
                l_t = lnt.tile([P, DT, P], BF, tag="lnt", name="lt")
                nc.scalar.copy(l_t, tp[:, 0:512].rearrange("p (c n) -> p c n", n=P))

                f_ps = ps_acc.tile([P, 512], FP, tag="acc", name="ffps")
                for c in range(DT):
                    nc.tensor.matmul(
                        f_ps,
                        l_t[:, c, :],
                        wsb["Wo"][:, c, :],
                        start=(c == 0),
                        stop=(c == DT - 1),
                    )
                o2 = otile.tile([P, D], BF, tag="o2", name="o2")
                if trivbo:
                    # o2 = relu(f) + ln1 in one DVE op
                    nc.vector.scalar_tensor_tensor(
                        out=o2, in0=f_ps, scalar=0.0, in1=ln1[:, q, :],
                        op0=ALU.max, op1=ALU.add,
                    )
                else:
                    rf = otile.tile([P, D], FP, tag="rf", name="rf")
                    nc.vector.tensor_tensor(rf, f_ps, bc["bo"], ALU.add)
                    nc.vector.scalar_tensor_tensor(
                        out=o2, in0=rf, scalar=0.0, in1=ln1[:, q, :],
                        op0=ALU.max, op1=ALU.add,
                    )
                st2 = sml.tile([P, 6], FP, tag="bn", name="st2")
                nc.vector.bn_stats(st2, o2)
                mv2 = sml.tile([P, 2], FP, tag="mv", name="mv2")
                nc.vector.bn_aggr(mv2, st2)
                ve2 = sml.tile([P, 1], FP, tag="ve2", name="ve2")
                nc.vector.tensor_scalar(
                    out=ve2, in0=mv2[:, 1:2], scalar1=EPS, scalar2=None,
                    op0=ALU.add,
                )
                rs2 = sml.tile([P, 1], FP, tag="rs", name="rs2")
                rsqrt_dve(rs2[:], ve2[:], sml, 1)
                z2 = otile.tile([P, D], FP, tag="z", name="z2")
                nc.gpsimd.tensor_scalar(
                    out=z2,
                    in0=o2,
                    scalar1=mv2[:, 0:1],
                    scalar2=rs2,
                    op0=ALU.subtract,
                    op1=ALU.mult,
                )
                if not triv1:
                    nc.vector.tensor_tensor(z2, z2, bc["g1"], ALU.mult)
                    nc.vector.tensor_tensor(z2, z2, bc["b1"], ALU.add)
                # un-permute D on the way out: dram dv = 256g+64a+32i+dl
                # sbuf j = 256g+128i+32a+dl
                dstq = out_O[b, ts(q, P), :]
                for g in range(2):
                    for i in range(2):
                        src = bass.AP(
                            tensor=z2.tensor,
                            offset=z2.offset + 256 * g + P * i,
                            ap=[list(z2.ap[0]), [32, 4], [1, 32]],
                        )
                        dst = bass.AP(
                            tensor=dstq.tensor,
                            offset=dstq.offset + 256 * g + 32 * i,
                            ap=[list(dstq.ap[0]), [64, 4], [1, 32]],
                        )
                        nc.sync.dma_start(out=dst, in_=src)

        phase_a(0)
        phase_b(0)
        phase_c_ln1(0)
        phase_c_ffn(0)
        phase_a(1)
        phase_b(1)
        phase_c_ln1(1)
        phase_c_ffn(1)

    nc.compile()
    return nc


_NC = {}


def _get_nc(triv0, triv1, trivbo):
    key = (triv0, triv1, trivbo)
    if key not in _NC:
        _NC[key] = _build_program(*key)
    return _NC[key]


def _perm():
    """Permuted-D order: position (g,i,a,dl) <- head 4g+a, dim 32i+dl."""
    p = np.empty(D, np.int64)
    for g in range(2):
        for i in range(2):
            for a in range(4):
                for dl in range(32):
                    tgt = (2 * g + i) * 128 + 32 * a + dl
                    src = 64 * (4 * g + a) + 32 * i + dl
                    p[tgt] = src
    return p


def _prep_in_maps(inputs):
    import ml_dtypes

    f32 = lambda x: np.ascontiguousarray(np.asarray(x), dtype=np.float32)
    bf = lambda x: np.ascontiguousarray(
        np.asarray(x, dtype=np.float32).astype(ml_dtypes.bfloat16)
    )
    perm = _perm()
    Q, K = f32(inputs["Q"]), f32(inputs["K"])
    QT = np.ascontiguousarray(Q.transpose(0, 2, 1))
    KT = np.ascontiguousarray(K.transpose(0, 2, 1))
    Wq = f32(inputs["Wq"])[:, perm]
    Wk = f32(inputs["Wk"])[:, perm]
    Wo = f32(inputs["Wo"])[perm][:, perm]
    shared = {
        "Wq": bf(Wq),
        "Wk": bf(Wk),
        "Wv": bf(inputs["Wv"]),
        "Wo": bf(Wo),
        "bq2": np.ascontiguousarray(f32(inputs["bq"])[perm].reshape(DT, P).T),
        "bk2": np.ascontiguousarray(f32(inputs["bk"])[perm].reshape(DT, P).T),
        "bv4": f32(inputs["bv"]) * 4.0,
        "bo": f32(inputs["bo"])[perm],
        "g0": f32(inputs["g0"])[perm],
        "b0": f32(inputs["b0"])[perm],
        "g1": f32(inputs["g1"])[perm],
        "b1": f32(inputs["b1"])[perm],
    }
    in_maps = []
    for c in range(NCORES):
        m = dict(shared)
        m["QT"] = np.ascontiguousarray(
            QT[c * BL : (c + 1) * BL].astype(ml_dtypes.bfloat16)
        )
        m["KT"] = np.ascontiguousarray(
            KT[c * BL : (c + 1) * BL].astype(ml_dtypes.bfloat16)
        )
        in_maps.append(m)
    return in_maps


def _run(inputs, trace=False):
    triv0 = bool(
        np.all(np.asarray(inputs["g0"]) == 1.0)
        and np.all(np.asarray(inputs["b0"]) == 0.0)
    )
    triv1 = bool(
        np.all(np.asarray(inputs["g1"]) == 1.0)
        and np.all(np.asarray(inputs["b1"]) == 0.0)
    )
    trivbo = bool(np.all(np.asarray(inputs["bo"]) == 0.0))
    nc = _get_nc(triv0, triv1, trivbo)
    in_maps = _prep_in_maps(inputs)
    return run_bass_kernel_spmd(nc, in_maps, list(range(NCORES)), trace=trace)


def kernel(**inputs):
    res = _run(inputs, trace=False)
    return np.concatenate([res.results[c]["O"] for c in range(NCORES)], axis=0)


# revision 13
# speedup vs baseline: 1.7495x; 1.7495x over previous
"""Trainium2 Bass kernel for the MAB (multihead attention block) problem.

Full inputs in, full outputs out. Data-parallel over batch: 16 batches
across 8 NeuronCores = 2 batches/core. No collectives.

Key techniques vs the bf16 baseline:
  * Attention matmuls (scores S^T = Kp Qp^T and PV) run in fp8e4 with
    MatmulPerfMode.DoubleRow -> 2 output columns/cycle (2x bf16 rate).
    Q/K are scaled by 4 before the fp8 cast (the 1/16 folds into the
    softmax exp scale); V is scaled by 4 with a 4.0-ones column so the
    denominator row cancels the factor exactly.
  * Scores layout: Wq/Wk columns are permuted on the host so the
    projection emits Qp^T/Kp^T with each head's 64 dims split as
    [32 partitions x 2 free planes] - the layout DoubleRow contracts
    over. All downstream tensors stay in this permuted-D order (LayerNorm
    is permutation invariant; Wo rows+cols are permuted to match) and the
    final output DMA un-permutes via a strided access pattern.
  * exp: split between the ACT engine (Exp activation, fp8e4 output,
    ~1 elem/lane/cycle) and the DVE (fast-exp: round(x*c1+c2) into a
    uint8 buffer whose bits are read as fp8e4; the softmax normalization
    cancels the systematic bias of the linear-mantissa approximation).
  * ACT runs only Exp-table functions (exp, copy); LN rsqrt is a
    Quake-style bit hack + 2 Newton steps on the DVE, so there are no
    activation-table swaps.
  * LN applies run on the otherwise-idle GpSimd engine (SBUF->SBUF),
    as does the Qp bf16 -> fp8 quantize.
  * FFN relu+residual is one fused DVE scalar_tensor_tensor
    ((f max 0) + ln1).
"""

import math
import sys
from contextlib import ExitStack

import numpy as np

sys.path.insert(0, "/opt/trn_rl_repo")

import concourse.bass as bass
import concourse.tile as tile
from concourse import bacc
from concourse import mybir
from concourse.bass import ds, ts
from concourse.bass_utils import run_bass_kernel_spmd
from concourse.masks import make_identity

FP = mybir.dt.float32
BF = mybir.dt.bfloat16
F8 = mybir.dt.float8e4
U8 = mybir.dt.uint8
I32 = mybir.dt.int32
AF = mybir.ActivationFunctionType
ALU = mybir.AluOpType
DR = mybir.MatmulPerfMode.DoubleRow

B, N, D = 16, 1024, 512
NCORES = 8
BL = B // NCORES  # batches per core
H, HD = 8, 64
SCALE = 1.0 / math.sqrt(D)
EPS = 1e-5
P = 128
DT = D // P  # 4 dv chunks
NT = N // P  # 8 nq/nk tiles
HA = HD + 1  # head dim + denominator column
HB = HA + 1  # 66: pad so bf16 PSUM blocks stay 4B-aligned
LOG2E = 1.4426950408889634
# fast-exp: fp8e4 bits of exp(x*SCALE/16) ~= round(x*FE_C1 + FE_C2) as u8
FE_C1 = 8.0 * LOG2E * (SCALE / 16.0)
FE_C2 = 55.6
# which (head, m) exp tiles run on DVE as fast-exp (the rest on ACT)
FASTEXP_DVE = {(h, 5) for h in range(H)} | {(1, 2), (5, 2)}


def _bcast_ap(ap):
    """Broadcast a [D]-shaped DRAM AP across all 128 partitions."""
    return bass.AP(tensor=ap.tensor, offset=ap.offset, ap=[[0, P]] + list(ap.ap))


def _free_bcast(tileap, inner):
    """[P, K] tile viewed as [P, K, inner] with stride-0 inner dim."""
    return bass.AP(
        tensor=tileap.tensor,
        offset=tileap.offset,
        ap=[list(tileap.ap[0]), list(tileap.ap[1]), [0, inner]],
    )


def _build_program(triv0, triv1, trivbo):
    nc = bacc.Bacc(None, target_bir_lowering=False)
    dr = {}
    for name, shape in [
        ("QT", [BL, D, N]),
        ("KT", [BL, D, N]),
        ("Wq", [D, D]),   # columns pre-permuted on host
        ("Wk", [D, D]),   # columns pre-permuted on host
        ("Wv", [D, D]),
        ("Wo", [D, D]),   # rows+columns pre-permuted on host
        ("bq2", [P, DT]),
        ("bq24", [P, DT]),
        ("bk2", [P, DT]),
        ("bv4", [D]),     # 4 * bv
        ("bo", [D]),      # permuted
        ("g0", [D]),      # permuted
        ("b0", [D]),      # permuted
        ("g1", [D]),      # permuted
        ("b1", [D]),      # permuted
    ]:
        dt = BF if name in ("QT", "KT", "Wq", "Wk", "Wv", "Wo") else FP
        dr[name] = nc.declare_dram_parameter(name, shape, dt, isOutput=False)
    out_O = nc.declare_dram_parameter("O", [BL, N, D], FP, isOutput=True)

    qt_src = dr["QT"][:].rearrange("b (c p) n -> b p c n", p=P)
    kt_src = dr["KT"][:].rearrange("b (c p) n -> b p c n", p=P)

    with tile.TileContext(nc) as tc, ExitStack() as ctx:
        singles = ctx.enter_context(tc.tile_pool(name="singles", bufs=1))
        work = ctx.enter_context(tc.tile_pool(name="work", bufs=1))
        pch = ctx.enter_context(tc.tile_pool(name="pch", bufs=3))
        lnt = ctx.enter_context(tc.tile_pool(name="lnt", bufs=2))
        ost = ctx.enter_context(tc.tile_pool(name="ost", bufs=2))
        otile = ctx.enter_context(tc.tile_pool(name="otile", bufs=2))
        sml = ctx.enter_context(tc.tile_pool(name="sml", bufs=8))
        # PSUM: flow 2x2 banks (score tiles) + opv 2 + acc 2x1 = 8 banks
        ps_acc = ctx.enter_context(tc.tile_pool(name="ps_acc", bufs=2, space="PSUM"))
        ps_pv = ctx.enter_context(tc.tile_pool(name="ps_pv", bufs=1, space="PSUM"))
        ps_flow = ctx.enter_context(tc.tile_pool(name="ps_flow", bufs=2, space="PSUM"))

        # ---- statics
        wsb = {}
        for wname in ("Wq", "Wk", "Wv", "Wo"):
            wsb[wname] = singles.tile([P, DT, D], BF, tag=wname, name=wname)
        nc.sync.dma_start(
            out=wsb["Wq"], in_=dr["Wq"][:].rearrange("(c p) d -> p c d", p=P)
        )
        bq_sb = singles.tile([P, DT], FP, tag="bq2")
        nc.sync.dma_start(out=bq_sb, in_=dr["bq2"][:])
        qb4_sb = singles.tile([P, DT], FP, tag="bq24")
        nc.sync.dma_start(out=qb4_sb, in_=dr["bq24"][:])
        bk_sb = singles.tile([P, DT], FP, tag="bk2")
        bc = {}
        for bname in ("bv4", "bo", "g0", "b0", "g1", "b1"):
            t = singles.tile([P, D], FP, tag=bname)
            nc.gpsimd.dma_start(out=t, in_=_bcast_ap(dr[bname][:]))
            bc[bname] = t
        ident = singles.tile([P, P], FP, tag="ident")
        make_identity(nc, ident)
        ident_b = singles.tile([P, P], BF, tag="identb")
        nc.vector.tensor_copy(ident_b, ident)

        state = {}

        def rsqrt_dve(out_ap, in_ap, scratch_pool, n):
            """out = 1/sqrt(in) via bit hack + 2 Newton iters. [P, n] fp32."""
            yh = scratch_pool.tile([P, n], FP, tag=f"rs_a{n}", name="rs_a")
            t0 = scratch_pool.tile([P, n], FP, tag=f"rs_b{n}", name="rs_b")
            nc.vector.tensor_scalar(
                out=yh[:].bitcast(I32), in0=in_ap.bitcast(I32),
                scalar1=1, scalar2=None, op0=ALU.logical_shift_right,
            )
            nc.vector.tensor_scalar(
                out=out_ap.bitcast(I32), in0=yh[:].bitcast(I32),
                scalar1=-1, scalar2=0x5F3759DF, op0=ALU.mult, op1=ALU.add,
            )
            for _ in range(2):
                nc.vector.tensor_tensor(t0, in_ap, out_ap, ALU.mult)
                nc.vector.tensor_tensor(t0, t0, out_ap, ALU.mult)
                nc.vector.tensor_scalar(
                    out=t0, in0=t0, scalar1=-0.5, scalar2=1.5,
                    op0=ALU.mult, op1=ALU.add,
                )
                nc.vector.tensor_tensor(out_ap, out_ap, t0, ALU.mult)

        def phase_a(b):
            st = {}
            qt = work.tile([P, DT, N], BF, tag="qt")
            kt = work.tile([P, DT, N], BF, tag="kt")
            for c in range(DT):
                nc.sync.dma_start(out=qt[:, c, :], in_=qt_src[b, :, c, :])
                nc.sync.dma_start(out=kt[:, c, :], in_=kt_src[b, :, c, :])
            if b == 0:
                for wname in ("Wk", "Wv", "Wo"):
                    nc.sync.dma_start(
                        out=wsb[wname],
                        in_=dr[wname][:].rearrange("(c p) d -> p c d", p=P),
                    )
                nc.sync.dma_start(out=bk_sb, in_=dr["bk2"][:])

            # Qp^T (bf16, for residual+fp8 quantize) and Kp^T (straight to fp8)
            qpt = work.tile([P, DT, N], BF, tag="qpt")
            ks = work.tile([P, DT, N], F8, tag="ks")
            qs = work.tile([P, DT, N], F8, tag="qs")
            for t in range(DT):
                ps = ps_flow.tile([P, N], FP, tag="flow", name="projq")
                for hf in range(2):
                    for c in range(DT):
                        nc.tensor.matmul(
                            ps[:, ds(hf * 512, 512)],
                            wsb["Wq"][:, c, ts(t, P)],
                            qt[:, c, ds(hf * 512, 512)],
                            start=(c == 0),
                            stop=(c == DT - 1),
                        )
                nc.vector.tensor_scalar_add(qpt[:, t, :], ps, bq_sb[:, t : t + 1])
                # fp8 copy (x4) for the score matmuls: Copy(4*ps + 4*bq) on ACT
                nc.scalar.activation(
                    qs[:, t, :], ps, AF.Identity, scale=4.0,
                    bias=qb4_sb[:, t : t + 1],
                )
            for t in range(DT):
                ps = ps_flow.tile([P, N], FP, tag="flow", name="projk")
                for hf in range(2):
                    for c in range(DT):
                        nc.tensor.matmul(
                            ps[:, ds(hf * 512, 512)],
                            wsb["Wk"][:, c, ts(t, P)],
                            kt[:, c, ds(hf * 512, 512)],
                            start=(c == 0),
                            stop=(c == DT - 1),
                        )
                # ks = (Kp + bk) * 4 -> fp8
                nc.vector.tensor_scalar(
                    out=ks[:, t, :], in0=ps, scalar1=bk_sb[:, t : t + 1],
                    scalar2=4.0, op0=ALU.add, op1=ALU.mult,
                )

            # Vp natural, augmented: per head 4*V (64 cols) + a 4.0 column
            vpa = work.tile([P, NT, H * HB], F8, tag="vpa")
            ones_ap = bass.AP(
                tensor=vpa.tensor, offset=vpa.offset + HD,
                ap=[list(vpa.ap[0]), [H * HB, NT], [HB, H], [1, 1]],
            )
            nc.gpsimd.memset(ones_ap, 4.0)

            def emit_vp(m):
                ps = ps_acc.tile([P, 512], FP, tag="acc", name="vps")
                for c in range(DT):
                    nc.tensor.matmul(
                        ps,
                        kt[:, c, ts(m, P)],
                        wsb["Wv"][:, c, :],
                        start=(c == 0),
                        stop=(c == DT - 1),
                    )
                vslice = bass.AP(
                    tensor=vpa.tensor, offset=vpa.offset + m * (H * HB),
                    ap=[list(vpa.ap[0]), [HB, H], [1, HD]],
                )
                # vpa = 4*Vp + 4*bv  (bv4 = 4*bv from host)
                nc.vector.scalar_tensor_tensor(
                    out=vslice,
                    in0=ps[:, :].rearrange("p (h s) -> p h s", s=HD),
                    scalar=4.0,
                    in1=bc["bv4"][:, :].rearrange("p (h s) -> p h s", s=HD),
                    op0=ALU.mult,
                    op1=ALU.add,
                )

            for m in range(4):
                emit_vp(m)
            st_vp_rest = [lambda m=m: emit_vp(m) for m in range(4, NT)]

            # Qp natural-permuted (for the attention residual), bf16
            qp = work.tile([P, NT, D], BF, tag="qp")

            st.update(qt=qt, kt=kt, qpt=qpt, qs=qs, ks=ks, vpa=vpa, qp=qp,
                      vp_rest=st_vp_rest)
            state[b] = st

        def qp_chunk(b, t):
            st = state[b]
            qpt, qp = st["qpt"], st["qp"]
            for half in range(2):
                tp = ps_acc.tile([P, 512], BF, tag="acc", name="qptr")
                for mm in range(4):
                    m = half * 4 + mm
                    nc.tensor.transpose(
                        tp[:, ts(mm, P)], qpt[:, t, ts(m, P)], ident_b
                    )
                nc.any.tensor_copy(
                    qp[:, ds(half * 4, 4), ts(t, P)],
                    tp.rearrange("p (mm n) -> p mm n", n=P),
                )

        def phase_b(b):
            st = state[b]
            qs, ks, vpa, qp = st["qs"], st["ks"], st["vpa"], st["qp"]
            oasm = work.tile([P, NT, D], BF, tag="oasm")
            pending_drain = [None]
            carry = []  # deferred PV matmul thunks crossing group boundary

            for h in range(H):
                g, a = h // 4, h % 4
                prow = ds(32 * a, 32)
                o_pair = ps_pv.tile([HA, N], FP, tag="opv")

                def emit_pv(t, p2, o_pair=o_pair, h=h):
                    lhs = bass.AP(
                        tensor=vpa.tensor,
                        offset=vpa.offset + 2 * t * (H * HB) + h * HB,
                        ap=[list(vpa.ap[0]), [H * HB, 2], [1, HA]],
                    )
                    for half in range(2):
                        nc.tensor.matmul(
                            o_pair[:, ds(half * 512, 512)],
                            lhs,
                            p2[:, :, ds(half * 512, 512)],
                            start=(t == 0),
                            stop=(t == 3),
                            perf_mode=DR,
                        )

                pend = []
                p2 = None
                for m in range(NT):
                    if m % 2 == 0:
                        p2 = pch.tile([P, 2, N], F8, tag="p2")
                    s = ps_flow.tile([P, N], FP, tag="flow", name="spair")
                    for half in range(2):
                        nc.tensor.matmul(
                            s[:, ds(half * 512, 512)],
                            ks[prow, ds(2 * g, 2), ts(m, P)],
                            qs[prow, ds(2 * g, 2), ds(half * 512, 512)],
                            start=True,
                            stop=True,
                            perf_mode=DR,
                            tile_position=(32 * a, 0),
                        )
                    if (h, m) in FASTEXP_DVE:
                        nc.vector.tensor_scalar(
                            out=p2[:, m % 2, :].bitcast(U8),
                            in0=s, scalar1=FE_C1, scalar2=FE_C2,
                            op0=ALU.mult, op1=ALU.add,
                        )
                    else:
                        nc.scalar.activation(
                            p2[:, m % 2, :], s, AF.Exp, scale=SCALE / 16.0
                        )
                    if carry:
                        carry.pop(0)()
                    if h == 0 and m in (0, 1):
                        for _ in range(2):
                            if st["vp_rest"]:
                                st["vp_rest"].pop(0)()
                    if m == 2 and pending_drain[0] is not None:
                        pending_drain[0]()
                        pending_drain[0] = None
                    if m % 2 == 1:
                        pend.append((m // 2, p2))
                        if len(pend) > 1:
                            emit_pv(*pend.pop(0))
                for t, pp in pend:
                    carry.append(lambda t=t, pp=pp, f=emit_pv: f(t, pp))
                pend = []

                # drain: PSUM -> SBUF bf16, transpose to natural, normalize by
                # the denominator row, add the Qp residual. Deferred into the
                # next group so the PE keeps streaming scores.
                def make_drain(o_pair=o_pair, h=h, g=g, a=a):
                    def drain():
                        o_sb = ost.tile([HA, N], BF, tag="ost", name="osb")
                        nc.vector.tensor_copy(o_sb, o_pair)
                        t_ps = ps_acc.tile([P, NT, HB], BF, tag="acc", name="otr")
                        for q in range(NT):
                            nc.tensor.transpose(
                                t_ps[:, q, 0:HA],
                                o_sb[:, ts(q, P)],
                                ident_b[0:HA, 0:HA],
                            )
                        r8 = sml.tile([P, NT], FP, tag="r8", name="r8")
                        den = bass.AP(
                            tensor=t_ps.tensor, offset=t_ps.offset + HD,
                            ap=[list(t_ps.ap[0]), [HB, NT]],
                        )
                        nc.vector.reciprocal(r8, den)
                        tmp = otile.tile([P, NT, HD], BF, tag="dtmp", name="dtmp")
                        nc.vector.tensor_tensor(
                            tmp,
                            bass.AP(
                                tensor=t_ps.tensor, offset=t_ps.offset,
                                ap=[list(t_ps.ap[0]), [HB, NT], [1, HD]],
                            ),
                            _free_bcast(r8[:], HD),
                            ALU.mult,
                        )
                        # head h columns in permuted-D order: two 32-blocks at
                        # 256g+32a and 256g+128+32a, per q tile
                        def hcols(tl):
                            return bass.AP(
                                tensor=tl.tensor,
                                offset=tl.offset + 256 * g + 32 * a,
                                ap=[list(tl.ap[0]), [D, NT], [P, 2], [1, 32]],
                            )
                        nc.vector.tensor_tensor(
                            hcols(oasm),
                            tmp[:].rearrange("p q (i e) -> p q i e", e=32),
                            hcols(qp),
                            ALU.add,
                        )
                    return drain

                pending_drain[0] = make_drain()
                if h == 0:
                    qp_chunk(b, 0)
                    qp_chunk(b, 1)
                elif h in (1, 2):
                    qp_chunk(b, h + 1)

            for f in carry:
                f()
            carry = []
            if pending_drain[0] is not None:
                pending_drain[0]()
                pending_drain[0] = None
            st.update(oasm=oasm)

        def phase_c_ln1(b):
            st = state[b]
            oasm = st["oasm"]
            ln1 = work.tile([P, NT, D], BF, tag="ln1", name="ln1")
            st["ln1"] = ln1
            mva = sml.tile([P, NT, 2], FP, tag="mva", name="mva")
            for q in range(NT):
                st_ = sml.tile([P, 6], FP, tag="bn", name="st")
                nc.vector.bn_stats(st_, oasm[:, q, :])
                nc.vector.bn_aggr(mva[:, q, :], st_)
            veps = sml.tile([P, NT], FP, tag="veps", name="veps")
            nc.vector.tensor_scalar(
                out=veps, in0=mva[:, :, 1], scalar1=EPS, scalar2=None,
                op0=ALU.add,
            )
            rsa = sml.tile([P, NT], FP, tag="rsa", name="rsa")
            rsqrt_dve(rsa[:], veps[:], sml, NT)
            for q in range(NT):
                lq = ln1[:, q, :]
                nc.vector.tensor_scalar(
                    out=lq,
                    in0=oasm[:, q, :],
                    scalar1=mva[:, q, 0:1],
                    scalar2=rsa[:, q : q + 1],
                    op0=ALU.subtract,
                    op1=ALU.mult,
                )
                if not triv0:
                    nc.vector.tensor_tensor(lq, lq, bc["g0"], ALU.mult)
                    nc.vector.tensor_tensor(lq, lq, bc["b0"], ALU.add)

        def phase_c_ffn(b):
            st = state[b]
            ln1 = st["ln1"]
            o2a = work.tile([P, NT, D], BF, tag="o2a", name="o2a")
            mv2a = sml.tile([P, NT, 2], FP, tag="mv2a", name="mv2a")
            for q in range(NT):
                lq = ln1[:, q, :]
                tp = ps_flow.tile([P, N], BF, tag="flow", name="lntr")
                for c in range(DT):
                    nc.tensor.transpose(tp[:, ts(c, P)], lq[:, ts(c, P)], ident_b)
                l_t = lnt.tile([P, DT, P], BF, tag="lnt", name="lt")
                nc.scalar.copy(l_t, tp[:, 0:512].rearrange("p (c n) -> p c n", n=P))

                f_ps = ps_acc.tile([P, 512], FP, tag="acc", name="ffps")
                for c in range(DT):
                    nc.tensor.matmul(
                        f_ps,
                        l_t[:, c, :],
                        wsb["Wo"][:, c, :],
                        start=(c == 0),
                        stop=(c == DT - 1),
                    )
                o2 = o2a[:, q, :]
                if trivbo:
                    # o2 = relu(f) + ln1 in one DVE op
                    nc.vector.scalar_tensor_tensor(
                        out=o2, in0=f_ps, scalar=0.0, in1=ln1[:, q, :],
                        op0=ALU.max, op1=ALU.add,
                    )
                else:
                    rf = otile.tile([P, D], FP, tag="rf", name="rf")
                    nc.vector.tensor_tensor(rf, f_ps, bc["bo"], ALU.add)
                    nc.vector.scalar_tensor_tensor(
                        out=o2, in0=rf, scalar=0.0, in1=ln1[:, q, :],
                        op0=ALU.max, op1=ALU.add,
                    )
                st2 = sml.tile([P, 6], FP, tag="bn", name="st2")
                nc.vector.bn_stats(st2, o2)
                nc.vector.bn_aggr(mv2a[:, q, :], st2)
            ve2 = sml.tile([P, NT], FP, tag="ve2", name="ve2")
            nc.vector.tensor_scalar(
                out=ve2, in0=mv2a[:, :, 1], scalar1=EPS, scalar2=None,
                op0=ALU.add,
            )
            rs2 = sml.tile([P, NT], FP, tag="rs", name="rs2")
            rsqrt_dve(rs2[:], ve2[:], sml, NT)
            for q in range(NT):
                z2 = otile.tile([P, D], FP, tag="z", name="z2")
                nc.vector.tensor_scalar(
                    out=z2,
                    in0=o2a[:, q, :],
                    scalar1=mv2a[:, q, 0:1],
                    scalar2=rs2[:, q : q + 1],
                    op0=ALU.subtract,
                    op1=ALU.mult,
                )
                if not triv1:
                    nc.vector.tensor_tensor(z2, z2, bc["g1"], ALU.mult)
                    nc.vector.tensor_tensor(z2, z2, bc["b1"], ALU.add)
                # un-permute D on the way out: dram dv = 256g+64a+32i+dl
                # sbuf j = 256g+128i+32a+dl
                dstq = out_O[b, ts(q, P), :]
                for g in range(2):
                    for i in range(2):
                        src = bass.AP(
                            tensor=z2.tensor,
                            offset=z2.offset + 256 * g + P * i,
                            ap=[list(z2.ap[0]), [32, 4], [1, 32]],
                        )
                        dst = bass.AP(
                            tensor=dstq.tensor,
                            offset=dstq.offset + 256 * g + 32 * i,
                            ap=[list(dstq.ap[0]), [64, 4], [1, 32]],
                        )
                        nc.sync.dma_start(out=dst, in_=src)

        phase_a(0)
        phase_b(0)
        phase_c_ln1(0)
        phase_c_ffn(0)
        phase_a(1)
        phase_b(1)
        phase_c_ln1(1)
        phase_c_ffn(1)

    nc.compile()
    return nc


_NC = {}


def _get_nc(triv0, triv1, trivbo):
    key = (triv0, triv1, trivbo)
    if key not in _NC:
        _NC[key] = _build_program(*key)
    return _NC[key]


def _perm():
    """Permuted-D order: position (g,i,a,dl) <- head 4g+a, dim 32i+dl."""
    p = np.empty(D, np.int64)
    for g in range(2):
        for i in range(2):
            for a in range(4):
                for dl in range(32):
                    tgt = (2 * g + i) * 128 + 32 * a + dl
                    src = 64 * (4 * g + a) + 32 * i + dl
                    p[tgt] = src
    return p


def _prep_in_maps(inputs):
    import ml_dtypes

    f32 = lambda x: np.ascontiguousarray(np.asarray(x), dtype=np.float32)
    bf = lambda x: np.ascontiguousarray(
        np.asarray(x, dtype=np.float32).astype(ml_dtypes.bfloat16)
    )
    perm = _perm()
    Q, K = f32(inputs["Q"]), f32(inputs["K"])
    QT = np.ascontiguousarray(Q.transpose(0, 2, 1))
    KT = np.ascontiguousarray(K.transpose(0, 2, 1))
    Wq = f32(inputs["Wq"])[:, perm]
    Wk = f32(inputs["Wk"])[:, perm]
    Wo = f32(inputs["Wo"])[perm][:, perm]
    shared = {
        "Wq": bf(Wq),
        "Wk": bf(Wk),
        "Wv": bf(inputs["Wv"]),
        "Wo": bf(Wo),
        "bq2": np.ascontiguousarray(f32(inputs["bq"])[perm].reshape(DT, P).T),
        "bq24": np.ascontiguousarray(4.0 * f32(inputs["bq"])[perm].reshape(DT, P).T),
        "bk2": np.ascontiguousarray(f32(inputs["bk"])[perm].reshape(DT, P).T),
        "bv4": f32(inputs["bv"]) * 4.0,
        "bo": f32(inputs["bo"])[perm],
        "g0": f32(inputs["g0"])[perm],
        "b0": f32(inputs["b0"])[perm],
        "g1": f32(inputs["g1"])[perm],
        "b1": f32(inputs["b1"])[perm],
    }
    in_maps = []
    for c in range(NCORES):
        m = dict(shared)
        m["QT"] = np.ascontiguousarray(
            QT[c * BL : (c + 1) * BL].astype(ml_dtypes.bfloat16)
        )
        m["KT"] = np.ascontiguousarray(
            KT[c * BL : (c + 1) * BL].astype(ml_dtypes.bfloat16)
        )
        in_maps.append(m)
    return in_maps


def _run(inputs, trace=False):
    triv0 = bool(
        np.all(np.asarray(inputs["g0"]) == 1.0)
        and np.all(np.asarray(inputs["b0"]) == 0.0)
    )
    triv1 = bool(
        np.all(np.asarray(inputs["g1"]) == 1.0)
        and np.all(np.asarray(inputs["b1"]) == 0.0)
    )
    trivbo = bool(np.all(np.asarray(inputs["bo"]) == 0.0))
    nc = _get_nc(triv0, triv1, trivbo)
    in_maps = _prep_in_maps(inputs)
    return run_bass_kernel_spmd(nc, in_maps, list(range(NCORES)), trace=trace)


def kernel(**inputs):
    res = _run(inputs, trace=False)
    return np.concatenate([res.results[c]["O"] for c in range(NCORES)], axis=0)


# revision 15
# speedup vs baseline: 2.3365x; 1.3355x over previous
"""Trainium2 Bass kernel for the MAB (multihead attention block) problem.

Full inputs in, full outputs out. Data-parallel over batch: 16 batches
across 8 NeuronCores = 2 batches/core. No collectives.

Vs the bf16 baseline:
  * PV matmuls run in fp8e4 with MatmulPerfMode.DoubleRow: one matmul
    consumes TWO nk-chunks (planes along the nk-tile axis) at the same
    per-instruction cost as one bf16 chunk (HW-verified 217ns vs 216ns)
    -> 2x on the PV half of attention. V is scaled by 4 (fp8 range) with
    a 4.0-ones column so the softmax denominator cancels the factor.
  * Scores stay bf16 (64-row j-alternating matmuls hide LDWEIGHTS;
    DoubleRow's 256-column weight load would double their cost).
  * exp writes fp8e4 P directly; a tunable subset of exp tiles runs on
    the DVE as a fast-exp (round(x*c1+c2) into a uint8 view of the fp8
    buffer; softmax normalization cancels the approximation bias).
  * ACT runs only Exp-set functions (Exp/Copy) - LN rsqrt is a bit-hack
    + 2 Newton steps on the DVE, so no activation-table swaps.
  * FFN relu+residual fused into one DVE scalar_tensor_tensor; LN2
    stats batched so the rsqrt chain runs once per batch.
  * Projections stream 1024 columns per matmul (2KB moving-operand cap).
"""

import math
import sys
from contextlib import ExitStack

import numpy as np

sys.path.insert(0, "/opt/trn_rl_repo")

import concourse.bass as bass
import concourse.tile as tile
from concourse import bacc
from concourse import mybir
from concourse.bass import ds, ts
from concourse.bass_utils import run_bass_kernel_spmd
from concourse.masks import make_identity

FP = mybir.dt.float32
BF = mybir.dt.bfloat16
F8 = mybir.dt.float8e4
U8 = mybir.dt.uint8
I32 = mybir.dt.int32
AF = mybir.ActivationFunctionType
ALU = mybir.AluOpType
DR = mybir.MatmulPerfMode.DoubleRow

B, N, D = 16, 1024, 512
NCORES = 8
BL = B // NCORES  # batches per core
H, HD = 8, 64
PAIRS = H // 2
SCALE = 1.0 / math.sqrt(D)
EPS = 1e-5
P = 128
DT = D // P  # 4 dv chunks
NT = N // P  # 8 nq/nk tiles
HA = HD + 1  # head dim + denominator column
HB = HA + 1  # 66: pad so bf16 PSUM blocks stay 4B-aligned
LOG2E = 1.4426950408889634
# fast-exp constants: fp8e4 bits of exp(x*SCALE) ~= round(x*FE_C1 + FE_C2)
FE_C1 = 8.0 * LOG2E * SCALE
FE_C2 = 55.6
# which (group, m) exp tiles run on DVE as fast-exp (group = hp*2+hf)
FASTEXP_DVE = {(gi, 5) for gi in range(8)} | {(1, 2), (5, 2)}


def _bcast_ap(ap):
    """Broadcast a [D]-shaped DRAM AP across all 128 partitions."""
    return bass.AP(tensor=ap.tensor, offset=ap.offset, ap=[[0, P]] + list(ap.ap))


def _free_bcast(tileap, inner):
    """[P, K] tile viewed as [P, K, inner] with stride-0 inner dim."""
    return bass.AP(
        tensor=tileap.tensor,
        offset=tileap.offset,
        ap=[list(tileap.ap[0]), list(tileap.ap[1]), [0, inner]],
    )


def _build_program(triv0, triv1, trivbo):
    nc = bacc.Bacc(None, target_bir_lowering=False)
    dr = {}
    for name, shape in [
        ("QT", [BL, D, N]),
        ("KT", [BL, D, N]),
        ("Wq", [D, D]),
        ("Wk", [D, D]),
        ("Wv", [D, D]),
        ("Wo", [D, D]),
        ("bq2", [P, DT]),
        ("bk2", [P, DT]),
        ("bv4", [D]),  # 4 * bv
        ("bo", [D]),
        ("g0", [D]),
        ("b0", [D]),
        ("g1", [D]),
        ("b1", [D]),
    ]:
        dt = BF if name in ("QT", "KT", "Wq", "Wk", "Wv", "Wo") else FP
        dr[name] = nc.declare_dram_parameter(name, shape, dt, isOutput=False)
    out_O = nc.declare_dram_parameter("O", [BL, N, D], FP, isOutput=True)

    qt_src = dr["QT"][:].rearrange("b (c p) n -> b p c n", p=P)
    kt_src = dr["KT"][:].rearrange("b (c p) n -> b p c n", p=P)

    with tile.TileContext(nc) as tc, ExitStack() as ctx:
        singles = ctx.enter_context(tc.tile_pool(name="singles", bufs=1))
        work = ctx.enter_context(tc.tile_pool(name="work", bufs=1))
        pch = ctx.enter_context(tc.tile_pool(name="pch", bufs=3))
        lnt = ctx.enter_context(tc.tile_pool(name="lnt", bufs=2))
        ost = ctx.enter_context(tc.tile_pool(name="ost", bufs=2))
        otile = ctx.enter_context(tc.tile_pool(name="otile", bufs=2))
        sml = ctx.enter_context(tc.tile_pool(name="sml", bufs=8))
        # PSUM: flow 2x2 banks (score tiles) + opv 2 + acc 2x1 = 8 banks
        ps_acc = ctx.enter_context(tc.tile_pool(name="ps_acc", bufs=2, space="PSUM"))
        ps_pv = ctx.enter_context(tc.tile_pool(name="ps_pv", bufs=1, space="PSUM"))
        ps_flow = ctx.enter_context(tc.tile_pool(name="ps_flow", bufs=2, space="PSUM"))

        # ---- statics
        wsb = {}
        for wname in ("Wq", "Wk", "Wv", "Wo"):
            wsb[wname] = singles.tile([P, DT, D], BF, tag=wname, name=wname)
        nc.sync.dma_start(
            out=wsb["Wq"], in_=dr["Wq"][:].rearrange("(c p) d -> p c d", p=P)
        )
        bq_sb = singles.tile([P, DT], FP, tag="bq2")
        nc.sync.dma_start(out=bq_sb, in_=dr["bq2"][:])
        bk_sb = singles.tile([P, DT], FP, tag="bk2")
        bc = {}
        for bname in ("bv4", "bo", "g0", "b0", "g1", "b1"):
            t = singles.tile([P, D], FP, tag=bname)
            nc.gpsimd.dma_start(out=t, in_=_bcast_ap(dr[bname][:]))
            bc[bname] = t
        ident = singles.tile([P, P], FP, tag="ident")
        make_identity(nc, ident)
        ident_b = singles.tile([P, P], BF, tag="identb")
        nc.vector.tensor_copy(ident_b, ident)

        state = {}

        def rsqrt_dve(out_ap, in_ap, n):
            """out = 1/sqrt(in) via bit hack + 2 Newton iters. [P, n] fp32."""
            yh = sml.tile([P, n], FP, tag=f"rs_a{n}", name="rs_a")
            t0 = sml.tile([P, n], FP, tag=f"rs_b{n}", name="rs_b")
            nc.vector.tensor_scalar(
                out=yh[:].bitcast(I32), in0=in_ap.bitcast(I32),
                scalar1=1, scalar2=None, op0=ALU.logical_shift_right,
            )
            nc.vector.tensor_scalar(
                out=out_ap.bitcast(I32), in0=yh[:].bitcast(I32),
                scalar1=-1, scalar2=0x5F3759DF, op0=ALU.mult, op1=ALU.add,
            )
            for _ in range(2):
                nc.vector.tensor_tensor(t0, in_ap, out_ap, ALU.mult)
                nc.vector.tensor_tensor(t0, t0, out_ap, ALU.mult)
                nc.vector.tensor_scalar(
                    out=t0, in0=t0, scalar1=-0.5, scalar2=1.5,
                    op0=ALU.mult, op1=ALU.add,
                )
                nc.vector.tensor_tensor(out_ap, out_ap, t0, ALU.mult)

        def phase_a(b):
            st = {}
            qt = work.tile([P, DT, N], BF, tag="qt")
            kt = work.tile([P, DT, N], BF, tag="kt")
            for c in range(DT):
                nc.sync.dma_start(out=qt[:, c, :], in_=qt_src[b, :, c, :])
                nc.sync.dma_start(out=kt[:, c, :], in_=kt_src[b, :, c, :])
            if b == 0:
                for wname in ("Wk", "Wv", "Wo"):
                    nc.sync.dma_start(
                        out=wsb[wname],
                        in_=dr[wname][:].rearrange("(c p) d -> p c d", p=P),
                    )
                nc.sync.dma_start(out=bk_sb, in_=dr["bk2"][:])

            qpt = work.tile([P, DT, N], BF, tag="qpt")
            kpt = work.tile([P, DT, N], BF, tag="kpt")
            for dst, w, bias, src in (
                (qpt, wsb["Wq"], bq_sb, qt),
                (kpt, wsb["Wk"], bk_sb, kt),
            ):
                for t in range(DT):
                    ps = ps_flow.tile([P, N], FP, tag="flow", name="projps")
                    for hf in range(2):
                        for c in range(DT):
                            nc.tensor.matmul(
                                ps[:, ds(hf * 512, 512)],
                                w[:, c, ts(t, P)],
                                src[:, c, ds(hf * 512, 512)],
                                start=(c == 0),
                                stop=(c == DT - 1),
                            )
                    nc.vector.tensor_scalar_add(dst[:, t, :], ps, bias[:, t : t + 1])

            # Vp natural, fp8, augmented: per head 4*V (64 cols) + a 4.0 column
            vpa = work.tile([P, NT, H * HB], F8, tag="vpa")
            ones_ap = bass.AP(
                tensor=vpa.tensor, offset=vpa.offset + HD,
                ap=[list(vpa.ap[0]), [H * HB, NT], [HB, H], [1, 1]],
            )
            nc.gpsimd.memset(ones_ap, 4.0)

            def emit_vp(m):
                ps = ps_acc.tile([P, 512], FP, tag="acc", name="vps")
                for c in range(DT):
                    nc.tensor.matmul(
                        ps,
                        kt[:, c, ts(m, P)],
                        wsb["Wv"][:, c, :],
                        start=(c == 0),
                        stop=(c == DT - 1),
                    )
                vslice = bass.AP(
                    tensor=vpa.tensor, offset=vpa.offset + m * (H * HB),
                    ap=[list(vpa.ap[0]), [HB, H], [1, HD]],
                )
                # vpa = 4*Vp + 4*bv  (bv4 = 4*bv from host)
                nc.vector.scalar_tensor_tensor(
                    out=vslice,
                    in0=ps[:, :].rearrange("p (h s) -> p h s", s=HD),
                    scalar=4.0,
                    in1=bc["bv4"][:, :].rearrange("p (h s) -> p h s", s=HD),
                    op0=ALU.mult,
                    op1=ALU.add,
                )

            for m in range(4):
                emit_vp(m)
            st_vp_rest = [lambda m=m: emit_vp(m) for m in range(4, NT)]

            qp = work.tile([P, NT, D], BF, tag="qp")

            st.update(qt=qt, kt=kt, qpt=qpt, kpt=kpt, vpa=vpa, qp=qp,
                      vp_rest=st_vp_rest)
            state[b] = st

        def qp_chunk(b, t):
            st = state[b]
            qpt, qp = st["qpt"], st["qp"]
            for half in range(2):
                tp = ps_acc.tile([P, 512], BF, tag="acc", name="qptr")
                for mm in range(4):
                    m = half * 4 + mm
                    nc.tensor.transpose(
                        tp[:, ts(mm, P)], qpt[:, t, ts(m, P)], ident_b
                    )
                nc.any.tensor_copy(
                    qp[:, ds(half * 4, 4), ts(t, P)],
                    tp.rearrange("p (mm n) -> p mm n", n=P),
                )

        def phase_b(b):
            st = state[b]
            gi = [0]
            qpt, kpt, vpa, qp = st["qpt"], st["kpt"], st["vpa"], st["qp"]
            oasm = work.tile([P, NT, D], BF, tag="oasm")
            pending_drain = [None]
            carry = []  # deferred PV matmul thunks crossing group boundary

            for hp in range(PAIRS):
                for hf in range(2):
                    qslice = ds(hf * 512, 512)
                    o_pair = ps_pv.tile([HA, N], FP, tag="opv")

                    def emit_pv(t, p2, o_pair=o_pair, hp=hp):
                        for j in range(2):
                            lhs = bass.AP(
                                tensor=vpa.tensor,
                                offset=vpa.offset
                                + 2 * t * (H * HB)
                                + (2 * hp + j) * HB,
                                ap=[list(vpa.ap[0]), [H * HB, 2], [1, HA]],
                            )
                            nc.tensor.matmul(
                                o_pair[:, ds(j * 512, 512)],
                                lhs,
                                p2[:, :, ds(j * 512, 512)],
                                start=(t == 0),
                                stop=(t == 3),
                                perf_mode=DR,
                            )

                    pend = []
                    p2 = None
                    for m in range(NT):
                        if m % 2 == 0:
                            p2 = pch.tile([P, 2, N], F8, tag="p2")
                        s_pair = ps_flow.tile([P, N], FP, tag="flow", name="spair")
                        for j in range(2):
                            lo = j * 64
                            nc.tensor.matmul(
                                s_pair[:, ds(j * 512, 512)],
                                kpt[lo : lo + 64, hp, ts(m, P)],
                                qpt[lo : lo + 64, hp, qslice],
                                start=True,
                                stop=True,
                            )
                        if (gi[0], m) in FASTEXP_DVE:
                            nc.vector.tensor_scalar(
                                out=p2[:, m % 2, :].bitcast(U8),
                                in0=s_pair, scalar1=FE_C1, scalar2=FE_C2,
                                op0=ALU.mult, op1=ALU.add,
                            )
                        else:
                            nc.scalar.activation(
                                p2[:, m % 2, :], s_pair, AF.Exp, scale=SCALE
                            )
                        if carry:
                            carry.pop(0)()
                        if gi[0] == 0 and m in (0, 1):
                            for _ in range(2):
                                if st["vp_rest"]:
                                    st["vp_rest"].pop(0)()
                        if m == 2 and pending_drain[0] is not None:
                            pending_drain[0]()
                            pending_drain[0] = None
                        if m % 2 == 1:
                            pend.append((m // 2, p2))
                            if len(pend) > 1:
                                emit_pv(*pend.pop(0))
                    for t, pp in pend:
                        carry.append(lambda t=t, pp=pp, f=emit_pv: f(t, pp))
                    pend = []

                    # drain: PSUM -> SBUF bf16, transpose to natural,
                    # normalize rows by 1/denominator, add the Qp residual.
                    # Deferred into the next group.
                    def make_drain(o_pair=o_pair, hp=hp, hf=hf):
                        def drain():
                            o_sb = ost.tile([HA, N], BF, tag="ost", name="osb")
                            nc.vector.tensor_copy(o_sb, o_pair)
                            t_ps = ps_acc.tile(
                                [P, 2 * DT, HB], BF, tag="acc", name="otr"
                            )
                            for blk in range(2 * DT):
                                nc.tensor.transpose(
                                    t_ps[:, blk, 0:HA],
                                    o_sb[:, ts(blk, P)],
                                    ident_b[0:HA, 0:HA],
                                )
                            r8 = sml.tile([P, 2 * DT], FP, tag="r8", name="r8")
                            den = bass.AP(
                                tensor=t_ps.tensor, offset=t_ps.offset + HD,
                                ap=[list(t_ps.ap[0]), [HB, 2 * DT]],
                            )
                            nc.vector.reciprocal(r8, den)
                            tmp = otile.tile(
                                [P, 2 * DT, HD], BF, tag="dtmp", name="dtmp"
                            )
                            nc.vector.tensor_tensor(
                                tmp,
                                bass.AP(
                                    tensor=t_ps.tensor, offset=t_ps.offset,
                                    ap=[list(t_ps.ap[0]), [HB, 2 * DT], [1, HD]],
                                ),
                                _free_bcast(r8[:], HD),
                                ALU.mult,
                            )
                            # blk = j*4+qq -> head 2hp+j, q tile hf*4+qq
                            def hcols(tl):
                                return bass.AP(
                                    tensor=tl.tensor,
                                    offset=tl.offset
                                    + (hf * 4) * D
                                    + (2 * hp) * HD,
                                    ap=[list(tl.ap[0]), [HD, 2], [D, DT], [1, HD]],
                                )
                            nc.vector.tensor_tensor(
                                hcols(oasm),
                                tmp[:].rearrange("p (j q) e -> p j q e", j=2),
                                hcols(qp),
                                ALU.add,
                            )
                        return drain

                    pending_drain[0] = make_drain()
                    if gi[0] == 0:
                        qp_chunk(b, 0)
                        qp_chunk(b, 1)
                    elif gi[0] in (1, 2):
                        qp_chunk(b, gi[0] + 1)
                    gi[0] += 1

            for f in carry:
                f()
            carry = []
            if pending_drain[0] is not None:
                pending_drain[0]()
                pending_drain[0] = None
            st.update(oasm=oasm)

        def phase_c_ln1(b):
            st = state[b]
            oasm = st["oasm"]
            ln1 = work.tile([P, NT, D], BF, tag="ln1", name="ln1")
            st["ln1"] = ln1
            mva = sml.tile([P, NT, 2], FP, tag="mva", name="mva")
            for q in range(NT):
                st_ = sml.tile([P, 6], FP, tag="bn", name="st")
                nc.vector.bn_stats(st_, oasm[:, q, :])
                nc.vector.bn_aggr(mva[:, q, :], st_)
            veps = sml.tile([P, NT], FP, tag="veps", name="veps")
            nc.vector.tensor_scalar(
                out=veps, in0=mva[:, :, 1], scalar1=EPS, scalar2=None, op0=ALU.add
            )
            rsa = sml.tile([P, NT], FP, tag="rsa", name="rsa")
            rsqrt_dve(rsa[:], veps[:], NT)
            for q in range(NT):
                lq = ln1[:, q, :]
                nc.vector.tensor_scalar(
                    out=lq,
                    in0=oasm[:, q, :],
                    scalar1=mva[:, q, 0:1],
                    scalar2=rsa[:, q : q + 1],
                    op0=ALU.subtract,
                    op1=ALU.mult,
                )
                if not triv0:
                    nc.vector.tensor_tensor(lq, lq, bc["g0"], ALU.mult)
                    nc.vector.tensor_tensor(lq, lq, bc["b0"], ALU.add)

        def phase_c_ffn(b):
            st = state[b]
            ln1 = st["ln1"]
            o2a = work.tile([P, NT, D], BF, tag="o2a", name="o2a")
            mv2a = sml.tile([P, NT, 2], FP, tag="mv2a", name="mv2a")
            for q in range(NT):
                lq = ln1[:, q, :]
                tp = ps_flow.tile([P, 512], BF, tag="flow", name="lntr")
                for c in range(DT):
                    nc.tensor.transpose(tp[:, ts(c, P)], lq[:, ts(c, P)], ident_b)
                l_t = lnt.tile([P, DT, P], BF, tag="lnt", name="lt")
                nc.scalar.copy(l_t, tp[:].rearrange("p (c n) -> p c n", n=P))

                f_ps = ps_acc.tile([P, 512], FP, tag="acc", name="ffps")
                for c in range(DT):
                    nc.tensor.matmul(
                        f_ps,
                        l_t[:, c, :],
                        wsb["Wo"][:, c, :],
                        start=(c == 0),
                        stop=(c == DT - 1),
                    )
                o2 = o2a[:, q, :]
                if trivbo:
                    nc.vector.scalar_tensor_tensor(
                        out=o2, in0=f_ps, scalar=0.0, in1=ln1[:, q, :],
                        op0=ALU.max, op1=ALU.add,
                    )
                else:
                    rf = otile.tile([P, D], FP, tag="rf", name="rf")
                    nc.vector.tensor_tensor(rf, f_ps, bc["bo"], ALU.add)
                    nc.vector.scalar_tensor_tensor(
                        out=o2, in0=rf, scalar=0.0, in1=ln1[:, q, :],
                        op0=ALU.max, op1=ALU.add,
                    )
                st2 = sml.tile([P, 6], FP, tag="bn", name="st2")
                nc.vector.bn_stats(st2, o2)
                nc.vector.bn_aggr(mv2a[:, q, :], st2)
            ve2 = sml.tile([P, NT], FP, tag="ve2", name="ve2")
            nc.vector.tensor_scalar(
                out=ve2, in0=mv2a[:, :, 1], scalar1=EPS, scalar2=None, op0=ALU.add
            )
            rs2 = sml.tile([P, NT], FP, tag="rs", name="rs2")
            rsqrt_dve(rs2[:], ve2[:], NT)
            for q in range(NT):
                z2 = otile.tile([P, D], FP, tag="z", name="z2")
                nc.vector.tensor_scalar(
                    out=z2,
                    in0=o2a[:, q, :],
                    scalar1=mv2a[:, q, 0:1],
                    scalar2=rs2[:, q : q + 1],
                    op0=ALU.subtract,
                    op1=ALU.mult,
                )
                if not triv1:
                    nc.vector.tensor_tensor(z2, z2, bc["g1"], ALU.mult)
                    nc.vector.tensor_tensor(z2, z2, bc["b1"], ALU.add)
                nc.sync.dma_start(out=out_O[b, ts(q, P), :], in_=z2)

        phase_a(0)
        phase_b(0)
        phase_c_ln1(0)
        phase_c_ffn(0)
        phase_a(1)
        phase_b(1)
        phase_c_ln1(1)
        phase_c_ffn(1)

    nc.compile()
    return nc


_NC = {}


def _get_nc(triv0, triv1, trivbo):
    key = (triv0, triv1, trivbo)
    if key not in _NC:
        _NC[key] = _build_program(*key)
    return _NC[key]


def _prep_in_maps(inputs):
    import ml_dtypes

    f32 = lambda x: np.ascontiguousarray(np.asarray(x), dtype=np.float32)
    bf = lambda x: np.ascontiguousarray(
        np.asarray(x, dtype=np.float32).astype(ml_dtypes.bfloat16)
    )
    Q, K = f32(inputs["Q"]), f32(inputs["K"])
    QT = np.ascontiguousarray(Q.transpose(0, 2, 1))
    KT = np.ascontiguousarray(K.transpose(0, 2, 1))
    shared = {
        "Wq": bf(inputs["Wq"]),
        "Wk": bf(inputs["Wk"]),
        "Wv": bf(inputs["Wv"]),
        "Wo": bf(inputs["Wo"]),
        "bq2": np.ascontiguousarray(f32(inputs["bq"]).reshape(DT, P).T),
        "bk2": np.ascontiguousarray(f32(inputs["bk"]).reshape(DT, P).T),
        "bv4": f32(inputs["bv"]) * 4.0,
        "bo": f32(inputs["bo"]),
        "g0": f32(inputs["g0"]),
        "b0": f32(inputs["b0"]),
        "g1": f32(inputs["g1"]),
        "b1": f32(inputs["b1"]),
    }
    in_maps = []
    for c in range(NCORES):
        m = dict(shared)
        m["QT"] = np.ascontiguousarray(
            QT[c * BL : (c + 1) * BL].astype(ml_dtypes.bfloat16)
        )
        m["KT"] = np.ascontiguousarray(
            KT[c * BL : (c + 1) * BL].astype(ml_dtypes.bfloat16)
        )
        in_maps.append(m)
    return in_maps


def _run(inputs, trace=False):
    triv0 = bool(
        np.all(np.asarray(inputs["g0"]) == 1.0)
        and np.all(np.asarray(inputs["b0"]) == 0.0)
    )
    triv1 = bool(
        np.all(np.asarray(inputs["g1"]) == 1.0)
        and np.all(np.asarray(inputs["b1"]) == 0.0)
    )
    trivbo = bool(np.all(np.asarray(inputs["bo"]) == 0.0))
    nc = _get_nc(triv0, triv1, trivbo)
    in_maps = _prep_in_maps(inputs)
    return run_bass_kernel_spmd(nc, in_maps, list(range(NCORES)), trace=trace)


def kernel(**inputs):
    res = _run(inputs, trace=False)
    return np.concatenate([res.results[c]["O"] for c in range(NCORES)], axis=0)
